# revision 16
# baseline (speedup 1.0000x reference)
"""Decoupled-RoPE MHA on 8 Trainium2 NeuronCores (Bass/Tile).

Sharding: batch x heads. Cores 0-3 handle batch 0, cores 4-7 batch 1; each
core owns 4 of the 16 heads. Per core:

  - All tensors live in transposed layouts so every matmul contracts over the
    partition dim: Q.T/K.T [dh, S] per head, V natural [S, dh].
  - RoPE is applied in the transposed layout with host-precomputed cos and
    sign-folded sin tables; rotate_half becomes a 32-partition swap done with
    two SBUF->SBUF DMAs.
  - Attention computes S.T = K.T.T @ Q.T directly (scores transposed), exp
    without max-shift (logits are O(1) here), P@V as V.T @ P.T via natural-V
    chunks, and the softmax denominator via a ones-column matmul. The 1/l
    normalization is applied to A.T columns after a partition-broadcast.
  - Causality: q-tiles are processed in groups of 4 (512 q cols); key chunks
    beyond each group's range are skipped, in-group diagonal blocks get an
    additive -1e30 mask before exp.
  - Output projection computes the partial out.T = wo_cols @ A.T; a
    ReduceScatter over each batch's 4 cores sums partials and leaves each
    core with a 512-row slice of out.T.

The host shards/transposes inputs, feeds per-core DRAM tensors, and
reassembles (out, k_heads, v_heads) to match the reference exactly.
"""

import math
import numpy as np
from contextlib import ExitStack

import concourse.bass as bass
import concourse.mybir as mybir
import concourse.tile as tile
from concourse.bass_utils import run_bass_kernel_spmd
from concourse.vector_clock import ScopedClock, VectorClock

F32 = mybir.dt.float32
MM_DT = mybir.dt.float32r  # full-rate fp32 matmul mode (N>=256)

B, S, D, H = 2, 2048, 2048, 16
dh = D // H            # 128
nope = dh // 2         # 64
rope = dh // 2         # 64
HPC = 4                # heads per core
NC = D // 128          # 16 contraction chunks
NSL = S // 512         # 4 column slices of S
NTOK = S // 128        # 16 token chunks
NG = 4                 # q-tile groups (512 q cols each)
SCALE = 1.0 / math.sqrt(dh)
NEG = -1.0e30
ROPE_THETA = 10000.0
NCORES = 8
GROUPS = [[0, 1, 2, 3], [4, 5, 6, 7]]


class SafeTileContext(tile.TileContext):
    """TileContext for a walrus build that allows only ONE sync wait per
    instruction: multi-wait instructions get single-wait NoOps hoisted in
    front of them, and the final drain is split into per-proc drains."""

    def _add_instruction(self, inst):
        si = inst.sync_info
        if si is not None and si.on_wait is not None and len(si.on_wait) > 1:
            waits = list(si.on_wait)
            for w in waits[:-1]:
                nop = mybir.InstNoOp(
                    name=self.nc.get_next_instruction_name(),
                    ins=[],
                    outs=[],
                    engine=inst.engine,
                    sync_info=mybir.SyncInfo(on_wait=[w], on_update=[]),
                )
                super()._add_instruction(nop)
            inst.sync_info = mybir.SyncInfo(
                on_wait=[waits[-1]], on_update=list(si.on_update or [])
            )
        super()._add_instruction(inst)

    def _drain_and_barrier(self, tick_clock, wait_clock):
        gvc = tick_clock.global_clock
        n = len(gvc)
        for p in range(n):
            t = gvc[p]
            if t == 0:
                continue
            vec = [0] * n
            vec[p] = t
            d = self.nc.sync.drain(fusable=False)
            wait_clock.add_sem_waits(d.ins, ScopedClock({None: VectorClock(vec)}))
        self.nc.sync.drain(fusable=False)

        self.nc.all_engine_barrier()
        assert self.sems is not None
        popped = self.nc._tile_sem_poison_stack.pop()
        assert popped is self._sem_poison
        self.nc.clear_and_free_semaphores(list(self.sems.allocated().values()))
        self.nc.all_engine_barrier()


def _r(ap):
    return ap  # tiles are natively MM_DT


def _build_program():
    nc = bass.Bass()

    xT_d = nc.dram_tensor("xT", [D, S], MM_DT, kind="ExternalInput")
    wq_d = nc.dram_tensor("wq", [D, HPC * dh], MM_DT, kind="ExternalInput")
    wkn_d = nc.dram_tensor("wkn", [D, HPC * nope], MM_DT, kind="ExternalInput")
    wkr_d = nc.dram_tensor("wkr", [D, rope], MM_DT, kind="ExternalInput")
    wv_d = nc.dram_tensor("wv", [D, HPC * dh], MM_DT, kind="ExternalInput")
    wo_d = nc.dram_tensor("wo", [HPC * dh, D], MM_DT, kind="ExternalInput")
    cos_d = nc.dram_tensor("cosT", [rope, S], F32, kind="ExternalInput")
    sin_d = nc.dram_tensor("sinT", [rope, S], F32, kind="ExternalInput")
    mask_d = nc.dram_tensor("maskT", [128, NG * 512], F32, kind="ExternalInput")
    id_d = nc.dram_tensor("ident", [128, 128], MM_DT, kind="ExternalInput")
    ones_d = nc.dram_tensor("ones", [128, 1], MM_DT, kind="ExternalInput")
    onesr_d = nc.dram_tensor("onesr", [1, 128], MM_DT, kind="ExternalInput")

    kout_d = nc.dram_tensor("k_loc", [HPC, S, dh], F32, kind="ExternalOutput")
    vout_d = nc.dram_tensor("v_loc", [HPC, S, dh], MM_DT, kind="ExternalOutput")
    rsout_d = nc.dram_tensor("out_t_rs", [D // 4, S], F32, kind="ExternalOutput")

    with SafeTileContext(nc) as tc, ExitStack() as top, \
            nc.allow_low_precision(reason="fp32r rounding is intentional"):
        # ---- persistent pools (whole kernel) ----
        persist = top.enter_context(tc.tile_pool(name="persist", bufs=1))
        qh_sb = [persist.tile([dh, S], MM_DT, tag=f"qh{h}", name=f"qh{h}") for h in range(HPC)]
        kh_sb = [persist.tile([dh, S], MM_DT, tag=f"kh{h}", name=f"kh{h}") for h in range(HPC)]
        cos_sb = persist.tile([dh, S], F32, tag="cos")
        sin_sb = persist.tile([dh, S], F32, tag="sin")
        mask_sb = persist.tile([128, NG * 512], F32, tag="mask")
        ident = persist.tile([128, 128], MM_DT, tag="ident")
        ones_sb = persist.tile([128, 1], MM_DT, tag="ones")
        onesr_sb = persist.tile([1, 128], MM_DT, tag="onesr")

        nc.sync.dma_start(cos_sb[nope:dh, :], cos_d.ap())
        nc.sync.dma_start(sin_sb[nope:dh, :], sin_d.ap())
        nc.sync.dma_start(mask_sb[:, :], mask_d.ap())
        nc.sync.dma_start(ident[:, :], id_d.ap())
        nc.sync.dma_start(ones_sb[:, :], ones_d.ap())
        nc.sync.dma_start(onesr_sb[:, :], onesr_d.ap())

        dram = top.enter_context(tc.tile_pool(name="dram", bufs=1, space="DRAM"))
        po_t = dram.tile([D, S], F32, tag="po")
        rs_t = dram.tile([D // 4, S], F32, tag="rs")

        wq_r = wq_d.ap().rearrange("(c p) m -> p c m", p=128)     # [128,16,512]
        wkn_r = wkn_d.ap().rearrange("(c p) m -> p c m", p=128)   # [128,16,256]
        wkr_r = wkr_d.ap().rearrange("(c p) m -> p c m", p=128)   # [128,16,64]
        wv_r = wv_d.ap().rearrange("(c p) m -> p c m", p=128)     # [128,16,512]
        wo_r = wo_d.ap().rearrange("(c p) m -> p c m", p=128)     # [128,4,2048]
        vout_r = vout_d.ap().rearrange("h (t p) d -> t p h d", p=128)  # [16,128,4,128]

        # ================= Phase 1: projections =================
        with ExitStack() as ph:
            wvp = ph.enter_context(tc.tile_pool(name="wv", bufs=1))
            wv_sb = wvp.tile([128, NC, HPC * dh], MM_DT, tag="wv")   # 32KB/part
            nc.sync.dma_start(wv_sb[:, :, :], wv_r)
            wkrp = ph.enter_context(tc.tile_pool(name="wkr", bufs=1))
            wkr_sb = wkrp.tile([128, NC, rope], MM_DT, tag="wkr")
            nc.sync.dma_start(wkr_sb[:, :, :], wkr_r)

            krrp = ph.enter_context(tc.tile_pool(name="krr", bufs=1))
            krr_sb = krrp.tile([dh, S], MM_DT, tag="krr")

            inner = ph.enter_context(ExitStack())
            wqp = inner.enter_context(tc.tile_pool(name="wqs", bufs=2))
            wknp = inner.enter_context(tc.tile_pool(name="wkns", bufs=2))
            xp = inner.enter_context(tc.tile_pool(name="xt", bufs=NC + 2))
            psp = inner.enter_context(tc.tile_pool(name="ps_proj", bufs=3, space="PSUM"))
            vst = inner.enter_context(tc.tile_pool(name="vstage", bufs=2))

            for s in range(NSL):
                sl = bass.ts(s, 512)
                xt = []
                for c in range(NC):
                    t = xp.tile([128, 512], MM_DT, tag="xt")
                    nc.sync.dma_start(t[:, :], xT_d.ap()[c * 128:(c + 1) * 128, sl])
                    xt.append(t)

                # V natural: 4 token tiles of [128 tok, 512 dh(4 heads)]
                for tt in range(4):
                    ps = psp.tile([128, 512], F32, tag="psp")
                    for c in range(NC):
                        nc.tensor.matmul(
                            ps[:, :], _r(xt[c][:, bass.ts(tt, 128)]),
                            _r(wv_sb[:, c, :]),
                            start=(c == 0), stop=(c == NC - 1))
                    vs = vst.tile([128, HPC, dh], MM_DT, tag="vs")
                    nc.scalar.copy(vs[:, :, :], ps[:, :])
                    nc.sync.dma_start(vout_r[s * 4 + tt], vs[:, :, :])

                # Q.T per head: [128, 512]
                for h in range(HPC):
                    wqt = wqp.tile([128, NC, dh], MM_DT, tag="wq")
                    nc.sync.dma_start(wqt[:, :, :], wq_r[:, :, h * dh:(h + 1) * dh])
                    ps = psp.tile([128, 512], F32, tag="psp")
                    for c in range(NC):
                        nc.tensor.matmul(
                            ps[:, :], _r(wqt[:, c, :]), _r(xt[c][:, :]),
                            start=(c == 0), stop=(c == NC - 1))
                    nc.scalar.copy(qh_sb[h][:, sl], ps[:, :])

                # Kn per head: M=64 at partitions 0:64
                for h in range(HPC):
                    wkt = wknp.tile([128, NC, nope], MM_DT, tag="wkn")
                    nc.sync.dma_start(wkt[:, :, :], wkn_r[:, :, h * nope:(h + 1) * nope])
                    ps = psp.tile([128, 512], F32, tag="psp")
                    for c in range(NC):
                        nc.tensor.matmul(
                            ps[0:nope, :], _r(wkt[:, c, :]), _r(xt[c][:, :]),
                            start=(c == 0), stop=(c == NC - 1))
                    nc.scalar.copy(kh_sb[h][0:nope, sl], ps[0:nope, :])

                # Kr shared: M=64 at partitions 0:64, lifted to 64:128 below
                ps = psp.tile([128, 512], F32, tag="psp")
                for c in range(NC):
                    nc.tensor.matmul(
                        ps[0:nope, :], _r(wkr_sb[:, c, :]), _r(xt[c][:, :]),
                        start=(c == 0), stop=(c == NC - 1))
                nc.scalar.copy(krr_sb[0:nope, sl], ps[0:nope, :])

            inner.close()  # free xt/weight/psum pools before rope + transposes

            # lift Kr rows 0:64 -> 64:128 (cross-partition via SBUF-SBUF DMA)
            nc.sync.dma_start(krr_sb[nope:dh, :], krr_sb[0:nope, :])

            # ---- RoPE (rows 64:128 of krr and each qh) ----
            shp = ph.enter_context(tc.tile_pool(name="shuf", bufs=2))
            half = rope // 2

            def rope_rows(t):
                sh = shp.tile([dh, S], MM_DT, tag="sh")
                nc.sync.dma_start(sh[nope:nope + half, :], t[nope + half:dh, :])
                nc.sync.dma_start(sh[nope + half:dh, :], t[nope:nope + half, :])
                nc.vector.tensor_mul(t[nope:dh, :], t[nope:dh, :], cos_sb[nope:dh, :])
                nc.vector.tensor_mul(sh[nope:dh, :], sh[nope:dh, :], sin_sb[nope:dh, :])
                nc.vector.tensor_add(t[nope:dh, :], t[nope:dh, :], sh[nope:dh, :])

            rope_rows(krr_sb)
            for h in range(HPC):
                nc.vector.tensor_copy(kh_sb[h][nope:dh, :], krr_sb[nope:dh, :])
                rope_rows(qh_sb[h])

            # ---- k_heads natural output: PE transposes ----
            pst = ph.enter_context(tc.tile_pool(name="ps_tr", bufs=2, space="PSUM"))
            kts = ph.enter_context(tc.tile_pool(name="kts", bufs=3))
            for h in range(HPC):
                for c in range(NTOK):
                    pt = pst.tile([128, 128], MM_DT, tag="pt")
                    nc.tensor.transpose(pt[:, :], kh_sb[h][:, bass.ts(c, 128)], ident[:, :])
                    kt = kts.tile([128, 128], F32, tag="kt")
                    nc.vector.tensor_copy(kt[:, :], pt[:, :])
                    nc.sync.dma_start(kout_d.ap()[h, c * 128:(c + 1) * 128, :], kt[:, :])

        # ================= Phase 2: attention =================
        atp = top.enter_context(tc.tile_pool(name="atp", bufs=1))
        at_sb = [atp.tile([dh, S], MM_DT, tag=f"at{h}", name=f"at{h}") for h in range(HPC)]
        with ExitStack() as ph:
            pss = ph.enter_context(tc.tile_pool(name="ps_s", bufs=3, space="PSUM"))
            pso = ph.enter_context(tc.tile_pool(name="ps_o", bufs=2, space="PSUM"))
            psl = ph.enter_context(tc.tile_pool(name="ps_l", bufs=2, space="PSUM"))
            expp = ph.enter_context(tc.tile_pool(name="expp", bufs=4))
            vp = ph.enter_context(tc.tile_pool(name="vload", bufs=6))
            lp = ph.enter_context(tc.tile_pool(name="lrow", bufs=1))
            pslb = ph.enter_context(tc.tile_pool(name="ps_lb", bufs=1, space="PSUM"))
            l_sb = [lp.tile([1, S], MM_DT, tag=f"l{h}", name=f"l{h}") for h in range(HPC)]

            for h in range(HPC):
                for g in range(NG):
                    qcols = bass.ds(g * 512, 512)
                    kmax = NG * (g + 1)
                    ps_o = pso.tile([128, 512], F32, tag="po")
                    ps_l = psl.tile([1, 512], F32, tag="pl")
                    for kj in range(kmax):
                        ps_s = pss.tile([128, 512], F32, tag="ps")
                        nc.tensor.matmul(
                            ps_s[:, :], _r(kh_sb[h][:, bass.ts(kj, 128)]),
                            _r(qh_sb[h][:, qcols]), start=True, stop=True)
                        if kj >= NG * g:
                            o = kj - NG * g
                            nc.vector.tensor_add(
                                ps_s[:, :], ps_s[:, :], mask_sb[:, bass.ts(o, 512)])
                        ex = expp.tile([128, 512], MM_DT, tag="ex")
                        nc.scalar.activation(
                            ex[:, :], ps_s[:, :],
                            mybir.ActivationFunctionType.Exp, scale=SCALE)
                        vt = vp.tile([128, dh], MM_DT, tag="vt")
                        nc.sync.dma_start(
                            vt[:, :], vout_d.ap()[h, kj * 128:(kj + 1) * 128, :])
                        nc.tensor.matmul(
                            ps_o[:, :], _r(vt[:, :]), _r(ex[:, :]),
                            start=(kj == 0), stop=(kj == kmax - 1))
                        nc.tensor.matmul(
                            ps_l[:, :], _r(ones_sb[:, :]), _r(ex[:, :]),
                            start=(kj == 0), stop=(kj == kmax - 1))
                    nc.scalar.copy(at_sb[h][:, qcols], ps_o[:, :])
                    nc.vector.tensor_copy(l_sb[h][:, qcols], ps_l[:, :])

                nc.vector.reciprocal(l_sb[h][:, :], l_sb[h][:, :])
                for sc in range(NSL):
                    scs = bass.ts(sc, 512)
                    ps_lb = pslb.tile([128, 512], F32, tag="plb")
                    nc.tensor.matmul(
                        ps_lb[:, :], onesr_sb[:, :], l_sb[h][:, scs],
                        start=True, stop=True)
                    nc.vector.tensor_mul(
                        at_sb[h][:, scs], at_sb[h][:, scs], ps_lb[:, :])

        # ================= Phase 3: output projection + RS =================
        with ExitStack() as ph:
            wop = ph.enter_context(tc.tile_pool(name="wo", bufs=1))
            wo_sb = wop.tile([128, HPC, D], MM_DT, tag="wo")
            nc.sync.dma_start(wo_sb[:, :, :], wo_r)
            psp = ph.enter_context(tc.tile_pool(name="ps_out", bufs=3, space="PSUM"))
            post = ph.enter_context(tc.tile_pool(name="post", bufs=3))
            for dblk in range(NC):
                for sc in range(NSL):
                    ps = psp.tile([128, 512], F32, tag="pp")
                    for c4 in range(HPC):
                        nc.tensor.matmul(
                            ps[:, :], _r(wo_sb[:, c4, dblk * 128:(dblk + 1) * 128]),
                            _r(at_sb[c4][:, bass.ts(sc, 512)]),
                            start=(c4 == 0), stop=(c4 == HPC - 1))
                    po_s = post.tile([128, 512], F32, tag="pos")
                    nc.scalar.copy(po_s[:, :], ps[:, :])
                    nc.sync.dma_start(
                        po_t[dblk * 128:(dblk + 1) * 128, bass.ts(sc, 512)], po_s[:, :])

            nc.gpsimd.collective_compute(
                "ReduceScatter",
                mybir.AluOpType.add,
                replica_groups=GROUPS,
                ins=[po_t.opt()],
                outs=[rs_t.opt()],
            )
            nc.sync.dma_start(rsout_d.ap()[:, :], rs_t[:, :])

    return nc


def _rope_tables():
    freqs = 1.0 / (ROPE_THETA ** (np.arange(0, dh, 2, dtype=np.float32) / dh))
    emb = np.arange(S, dtype=np.float32)[:, None] * freqs[None, :]   # [S, 64]
    cos = np.tile(np.cos(emb)[:, : rope // 2], (1, 2)).T.astype(np.float32)
    sin = np.tile(np.sin(emb)[:, : rope // 2], (1, 2)).T.astype(np.float32)
    sin_signed = sin.copy()
    sin_signed[: rope // 2] *= -1.0
    return np.ascontiguousarray(cos), np.ascontiguousarray(sin_signed)


def _mask_table():
    kk = np.arange(128)[:, None]
    cc = np.arange(512)[None, :]
    cols = [np.where(o * 128 + kk <= cc, 0.0, NEG).astype(np.float32)
            for o in range(NG)]
    return np.ascontiguousarray(np.concatenate(cols, axis=1))


_NC_CACHE = None


def kernel(x, qkv, wk, wo):
    global _NC_CACHE
    if _NC_CACHE is None:
        _NC_CACHE = _build_program()
    nc = _NC_CACHE

    x = np.asarray(x, dtype=np.float32)
    qkv = np.asarray(qkv, dtype=np.float32)
    wk = np.asarray(wk, dtype=np.float32)
    wo = np.asarray(wo, dtype=np.float32)

    cos_t, sin_t = _rope_tables()
    mask_t = _mask_table()

    in_maps = []
    for core in range(NCORES):
        b = core // 4
        hs = (core % 4) * HPC
        in_maps.append({
            "xT": np.ascontiguousarray(x[b].T),
            "wq": np.ascontiguousarray(qkv[hs * dh:(hs + HPC) * dh].T),
            "wkn": np.ascontiguousarray(wk[hs * nope:(hs + HPC) * nope].T),
            "wkr": np.ascontiguousarray(wk[H * nope:].T),
            "wv": np.ascontiguousarray(qkv[D + hs * dh:D + (hs + HPC) * dh].T),
            "wo": np.ascontiguousarray(wo[:, hs * dh:(hs + HPC) * dh].T),
            "cosT": cos_t,
            "sinT": sin_t,
            "maskT": mask_t,
            "ident": np.eye(128, dtype=np.float32),
            "ones": np.ones((128, 1), dtype=np.float32),
            "onesr": np.ones((1, 128), dtype=np.float32),
        })

    res = run_bass_kernel_spmd(nc, in_maps, list(range(NCORES))).results

    out = np.empty((B, S, D), dtype=np.float32)
    k_heads = np.empty((B, H, S, dh), dtype=np.float32)
    v_heads = np.empty((B, H, S, dh), dtype=np.float32)
    for b in range(B):
        poT = np.concatenate(
            [res[b * 4 + r]["out_t_rs"] for r in range(4)], axis=0)   # [D, S]
        out[b] = poT.T
        for r in range(4):
            hs = r * HPC
            k_heads[b, hs:hs + HPC] = res[b * 4 + r]["k_loc"]
            v_heads[b, hs:hs + HPC] = res[b * 4 + r]["v_loc"]
    return out, k_heads, v_heads


# revision 38
# speedup vs baseline: 1.3693x; 1.3693x over previous
"""Decoupled-RoPE MHA on 8 Trainium2 NeuronCores (Bass/Tile).

Sharding: batch x heads. Cores 0-3 handle batch 0, cores 4-7 batch 1; each
core owns 4 of the 16 heads. Per core:

  - All tensors live in transposed layouts so every matmul contracts over the
    partition dim: Q.T/K.T [dh, S] per head, V natural [S, dh].
  - RoPE is applied in the transposed layout with host-precomputed cos and
    sign-folded sin tables; rotate_half becomes a 32-partition swap done with
    two SBUF->SBUF DMAs.
  - Attention computes S.T = K.T.T @ Q.T directly (scores transposed), exp
    without max-shift (logits are O(1) here), P@V as V.T @ P.T via natural-V
    chunks, and the softmax denominator via a ones-column matmul. The 1/l
    normalization is applied to A.T columns after a partition-broadcast.
  - Causality: q-tiles are processed in groups of 4 (512 q cols); key chunks
    beyond each group's range are skipped, in-group diagonal blocks get an
    additive -1e30 mask before exp.
  - Output projection computes the partial out.T = wo_cols @ A.T; a
    ReduceScatter over each batch's 4 cores sums partials and leaves each
    core with a 512-row slice of out.T.

The host shards/transposes inputs, feeds per-core DRAM tensors, and
reassembles (out, k_heads, v_heads) to match the reference exactly.
"""

import math
import numpy as np
from contextlib import ExitStack

import concourse.bass as bass
import concourse.mybir as mybir
import concourse.tile as tile
from concourse.bass_utils import run_bass_kernel_spmd
from concourse.vector_clock import ScopedClock, VectorClock

F32 = mybir.dt.float32
MM_DT = mybir.dt.float32r  # full-rate fp32 matmul mode (N>=256)

B, S, D, H = 2, 2048, 2048, 16
dh = D // H            # 128
nope = dh // 2         # 64
rope = dh // 2         # 64
HPC = 4                # heads per core
NC = D // 128          # 16 contraction chunks
NSL = S // 512         # 4 column slices of S
NTOK = S // 128        # 16 token chunks
NG = 4                 # q-tile groups (512 q cols each)
SCALE = 1.0 / math.sqrt(dh)
NEG = -1.0e30
ROPE_THETA = 10000.0
NCORES = 8
GROUPS = [[0, 1, 2, 3], [4, 5, 6, 7]]


class SafeTileContext(tile.TileContext):
    """TileContext for a walrus build that allows only ONE sync wait per
    instruction: multi-wait instructions get single-wait NoOps hoisted in
    front of them, and the final drain is split into per-proc drains."""

    def _add_instruction(self, inst):
        si = inst.sync_info
        if si is not None and si.on_wait is not None and len(si.on_wait) > 1:
            waits = list(si.on_wait)
            for w in waits[:-1]:
                nop = mybir.InstNoOp(
                    name=self.nc.get_next_instruction_name(),
                    ins=[],
                    outs=[],
                    engine=inst.engine,
                    sync_info=mybir.SyncInfo(on_wait=[w], on_update=[]),
                )
                super()._add_instruction(nop)
            inst.sync_info = mybir.SyncInfo(
                on_wait=[waits[-1]], on_update=list(si.on_update or [])
            )
        super()._add_instruction(inst)

    def _drain_and_barrier(self, tick_clock, wait_clock):
        gvc = tick_clock.global_clock
        n = len(gvc)
        for p in range(n):
            t = gvc[p]
            if t == 0:
                continue
            vec = [0] * n
            vec[p] = t
            d = self.nc.sync.drain(fusable=False)
            wait_clock.add_sem_waits(d.ins, ScopedClock({None: VectorClock(vec)}))
        self.nc.sync.drain(fusable=False)

        self.nc.all_engine_barrier()
        assert self.sems is not None
        popped = self.nc._tile_sem_poison_stack.pop()
        assert popped is self._sem_poison
        self.nc.clear_and_free_semaphores(list(self.sems.allocated().values()))
        self.nc.all_engine_barrier()


def _r(ap):
    return ap  # tiles are natively MM_DT


def _build_program():
    nc = bass.Bass()

    xT_d = nc.dram_tensor("xT", [D, S], MM_DT, kind="ExternalInput")
    wq_d = nc.dram_tensor("wq", [D, HPC * dh], MM_DT, kind="ExternalInput")
    wkn_d = nc.dram_tensor("wkn", [D, HPC * nope], MM_DT, kind="ExternalInput")
    wkr_d = nc.dram_tensor("wkr", [D, rope], MM_DT, kind="ExternalInput")
    wv_d = nc.dram_tensor("wv", [D, HPC * dh], MM_DT, kind="ExternalInput")
    wo_d = nc.dram_tensor("wo", [HPC * dh, D], MM_DT, kind="ExternalInput")
    cos_d = nc.dram_tensor("cosT", [rope, S], F32, kind="ExternalInput")
    sin_d = nc.dram_tensor("sinT", [rope, S], F32, kind="ExternalInput")
    mask_d = nc.dram_tensor("maskT", [128, NG * 512], F32, kind="ExternalInput")
    id_d = nc.dram_tensor("ident", [128, 128], MM_DT, kind="ExternalInput")
    ones_d = nc.dram_tensor("ones", [128, 1], MM_DT, kind="ExternalInput")
    onesr_d = nc.dram_tensor("onesr", [1, 128], MM_DT, kind="ExternalInput")

    kout_d = nc.dram_tensor("k_loc", [HPC, S, dh], F32, kind="ExternalOutput")
    vout_d = nc.dram_tensor("v_loc", [HPC, S, dh], MM_DT, kind="ExternalOutput")
    rsout_d = nc.dram_tensor("out_t_rs", [D // 4, S], F32, kind="ExternalOutput")

    with SafeTileContext(nc) as tc, ExitStack() as top, \
            nc.allow_low_precision(reason="fp32r rounding is intentional"):
        # ---- persistent pools (whole kernel) ----
        persist = top.enter_context(tc.tile_pool(name="persist", bufs=1))
        qh_sb = [persist.tile([dh, S], MM_DT, tag=f"qh{h}", name=f"qh{h}") for h in range(HPC)]
        kh_sb = [persist.tile([dh, S], MM_DT, tag=f"kh{h}", name=f"kh{h}") for h in range(HPC)]
        ident = persist.tile([128, 128], MM_DT, tag="ident")
        ones_sb = persist.tile([128, 1], MM_DT, tag="ones")
        onesr_sb = persist.tile([1, 128], MM_DT, tag="onesr")

        nc.sync.dma_start(ident[:, :], id_d.ap())
        nc.sync.dma_start(ones_sb[:, :], ones_d.ap())
        nc.sync.dma_start(onesr_sb[:, :], onesr_d.ap())

        dram = top.enter_context(tc.tile_pool(name="dram", bufs=1, space="DRAM"))
        po_sc = [dram.tile([D, 512], F32, tag=f"po{sc}", name=f"po{sc}")
                 for sc in range(NSL)]
        rs_sc = [dram.tile([D // 4, 512], F32, tag=f"rs{sc}", name=f"rs{sc}")
                 for sc in range(NSL)]

        wq_r = wq_d.ap().rearrange("(c p) m -> p c m", p=128)     # [128,16,512]
        wkn_r = wkn_d.ap().rearrange("(c p) m -> p c m", p=128)   # [128,16,256]
        wkr_r = wkr_d.ap().rearrange("(c p) m -> p c m", p=128)   # [128,16,64]
        wv_r = wv_d.ap().rearrange("(c p) m -> p c m", p=128)     # [128,16,512]
        wo_r = wo_d.ap().rearrange("(c p) m -> p c m", p=128)     # [128,4,2048]
        vout_r = vout_d.ap().rearrange("h (t p) d -> t p h d", p=128)  # [16,128,4,128]

        # ================= Phase 1: projections =================
        with ExitStack() as ph:
            wvp = ph.enter_context(tc.tile_pool(name="wv", bufs=1))
            wv_sb = wvp.tile([128, NC, HPC * dh], MM_DT, tag="wv")   # 32KB/part
            for c in range(NC):
                nc.sync.dma_start(wv_sb[:, c, :], wv_r[:, c, :])
            wkrp = ph.enter_context(tc.tile_pool(name="wkr", bufs=1))
            wkr_sb = wkrp.tile([128, NC, rope], MM_DT, tag="wkr")
            nc.sync.dma_start(wkr_sb[:, :, :], wkr_r)

            krrp = ph.enter_context(tc.tile_pool(name="krr", bufs=1))
            krr_sb = krrp.tile([dh, S], MM_DT, tag="krr")
            csp = ph.enter_context(tc.tile_pool(name="cstab", bufs=1))
            cos_sb = csp.tile([dh, S], F32, tag="cos")
            sin_sb = csp.tile([dh, S], F32, tag="sin")
            nc.sync.dma_start(cos_sb[nope:dh, :], cos_d.ap())
            nc.sync.dma_start(sin_sb[nope:dh, :], sin_d.ap())

            shp = ph.enter_context(tc.tile_pool(name="shuf", bufs=1))
            inner = ph.enter_context(ExitStack())
            wqp = inner.enter_context(tc.tile_pool(name="wqs", bufs=1))
            wq_sb = wqp.tile([128, NC, HPC * dh], MM_DT, tag="wqall")
            for c in range(NC):
                nc.sync.dma_start(wq_sb[:, c, :], wq_r[:, c, :])
            wknp = inner.enter_context(tc.tile_pool(name="wkns", bufs=1))
            xp = inner.enter_context(tc.tile_pool(name="xt", bufs=NC))
            psp = inner.enter_context(tc.tile_pool(name="ps_proj", bufs=3, space="PSUM"))
            vst = inner.enter_context(tc.tile_pool(name="vstage", bufs=2))

            half = rope // 2

            def rope_rows(t, sl):
                # roped = t*cos + shuffle(t)*sin_signed on rows 64:128
                sh = shp.tile([dh, 512], MM_DT, tag="sh")
                nc.sync.dma_start(sh[nope:nope + half, :], t[nope + half:dh, sl])
                nc.sync.dma_start(sh[nope + half:dh, :], t[nope:nope + half, sl])
                nc.vector.tensor_mul(t[nope:dh, sl], t[nope:dh, sl], cos_sb[nope:dh, sl])
                nc.vector.tensor_mul(sh[nope:dh, :], sh[nope:dh, :], sin_sb[nope:dh, sl])
                nc.vector.tensor_add(t[nope:dh, sl], t[nope:dh, sl], sh[nope:dh, :])

            for s in range(NSL):
                sl = bass.ts(s, 512)
                xt = []
                for c in range(NC):
                    t = xp.tile([128, 512], MM_DT, tag="xt")
                    nc.scalar.dma_start(t[:, :], xT_d.ap()[c * 128:(c + 1) * 128, sl])
                    xt.append(t)

                # V natural: 4 token tiles of [128 tok, 512 dh(4 heads)]
                for tt in range(4):
                    ps = psp.tile([128, 512], F32, tag="psp")
                    for c in range(NC):
                        nc.tensor.matmul(
                            ps[:, :], _r(xt[c][:, bass.ts(tt, 128)]),
                            _r(wv_sb[:, c, :]),
                            start=(c == 0), stop=(c == NC - 1))
                    vs = vst.tile([128, HPC, dh], MM_DT, tag="vs")
                    nc.scalar.copy(vs[:, :, :], ps[:, :])
                    nc.sync.dma_start(vout_r[s * 4 + tt], vs[:, :, :])

                # Q.T per head: [128, 512], rope applied per slice
                for h in range(HPC):
                    ps = psp.tile([128, 512], F32, tag="psp")
                    for c in range(NC):
                        nc.tensor.matmul(
                            ps[:, :], _r(wq_sb[:, c, h * dh:(h + 1) * dh]),
                            _r(xt[c][:, :]),
                            start=(c == 0), stop=(c == NC - 1))
                    nc.scalar.copy(qh_sb[h][:, sl], ps[:, :])
                    rope_rows(qh_sb[h], sl)

                # Kn head pairs: M=128 (rows 0:64 head 2p, 64:128 head 2p+1);
                # upper half reaches its kh rows via a PSUM->SBUF lift DMA
                for p in range(HPC // 2):
                    wkt = wknp.tile([128, NC, 2 * nope], MM_DT, tag="wkn")
                    nc.scalar.dma_start(
                        wkt[:, :, :], wkn_r[:, :, p * dh:(p + 1) * dh])
                    ps = psp.tile([128, 512], F32, tag="psp")
                    for c in range(NC):
                        nc.tensor.matmul(
                            ps[:, :], _r(wkt[:, c, :]), _r(xt[c][:, :]),
                            start=(c == 0), stop=(c == NC - 1))
                    nc.scalar.copy(kh_sb[2 * p][0:nope, sl], ps[0:nope, :])
                    kl = vst.tile([128, 512], MM_DT, tag="klift")
                    nc.scalar.copy(kl[nope:dh, :], ps[nope:dh, :])
                    nc.sync.dma_start(kh_sb[2 * p + 1][0:nope, sl], kl[nope:dh, :])

                # Kr shared: M=64 at partitions 0:64, lift + rope per slice
                ps = psp.tile([128, 512], F32, tag="psp")
                for c in range(NC):
                    nc.tensor.matmul(
                        ps[0:nope, :], _r(wkr_sb[:, c, :]), _r(xt[c][:, :]),
                        start=(c == 0), stop=(c == NC - 1))
                nc.scalar.copy(krr_sb[0:nope, sl], ps[0:nope, :])
                nc.sync.dma_start(krr_sb[nope:dh, sl], krr_sb[0:nope, sl])
                rope_rows(krr_sb, sl)
                for h in range(HPC):
                    nc.vector.tensor_copy(kh_sb[h][nope:dh, sl], krr_sb[nope:dh, sl])

            inner.close()  # free xt/weight/psum pools before transposes

            # ---- k_heads natural output: PE transposes ----
            pst = ph.enter_context(tc.tile_pool(name="ps_tr", bufs=2, space="PSUM"))
            kts = ph.enter_context(tc.tile_pool(name="kts", bufs=3))
            for h in range(HPC):
                for c in range(NTOK):
                    pt = pst.tile([128, 128], MM_DT, tag="pt")
                    nc.tensor.transpose(pt[:, :], kh_sb[h][:, bass.ts(c, 128)], ident[:, :])
                    kt = kts.tile([128, 128], F32, tag="kt")
                    nc.vector.tensor_copy(kt[:, :], pt[:, :])
                    nc.scalar.dma_start(kout_d.ap()[h, c * 128:(c + 1) * 128, :], kt[:, :])

        # ================= Phase 2: attention =================
        atp = top.enter_context(tc.tile_pool(name="atp", bufs=1))
        at_sb = [atp.tile([dh, S], MM_DT, tag=f"at{h}", name=f"at{h}") for h in range(HPC)]
        mask_sb = atp.tile([128, NG * 512], F32, tag="mask")
        nc.sync.dma_start(mask_sb[:, :], mask_d.ap())
        with ExitStack() as ph:
            pss = ph.enter_context(tc.tile_pool(name="ps_s", bufs=2, space="PSUM"))
            pso = ph.enter_context(tc.tile_pool(name="ps_o", bufs=2, space="PSUM"))
            psl = ph.enter_context(tc.tile_pool(name="ps_l", bufs=1, space="PSUM"))
            pslb = ph.enter_context(tc.tile_pool(name="ps_lb", bufs=1, space="PSUM"))
            psp3 = ph.enter_context(tc.tile_pool(name="ps_out", bufs=2, space="PSUM"))
            expp = ph.enter_context(tc.tile_pool(name="expp", bufs=9))
            vp = ph.enter_context(tc.tile_pool(name="vload", bufs=1))
            lp = ph.enter_context(tc.tile_pool(name="lrow", bufs=3))
            post = ph.enter_context(tc.tile_pool(name="post", bufs=3))
            wop = ph.enter_context(tc.tile_pool(name="wo", bufs=1))
            wo_sb = wop.tile([128, HPC, D], MM_DT, tag="wo")
            for c4 in range(HPC):
                nc.scalar.dma_start(wo_sb[:, c4, :], wo_r[:, c4, :])
            vout_c = vout_d.ap().rearrange("h (c p) d -> h p c d", p=128)
            # all V chunks for all 4 heads resident (32KB/part)
            v_sb = [vp.tile([128, NTOK, dh], MM_DT, tag=f"vsb{h}", name=f"vsb{h}")
                    for h in range(HPC)]
            for h in range(HPC):
                nc.gpsimd.dma_start(v_sb[h][:, :, :], vout_c[h])

            # g-outer: group g's attention for all heads, then its out-proj
            # column chunk and its ReduceScatter — collectives overlap the
            # next group's compute.
            for g in range(NG):
                qcols = bass.ds(g * 512, 512)
                kmax = NG * (g + 1)
                LOOK = 6
                for h in range(HPC):
                    # software-pipelined: scores+exp lead PV/l by LOOK chunks
                    # so PE never waits on ACT exp
                    ex_tiles = [None] * kmax
                    ps_o = pso.tile([128, 512], F32, tag="po")
                    ps_l = psl.tile([1, 512], F32, tag="pl")
                    for kj in range(kmax + LOOK):
                        if kj < kmax:
                            ps_s = pss.tile([128, 512], F32, tag="ps")
                            nc.tensor.matmul(
                                ps_s[:, :], _r(kh_sb[h][:, bass.ts(kj, 128)]),
                                _r(qh_sb[h][:, qcols]), start=True, stop=True)
                            if kj >= NG * g:
                                o = kj - NG * g
                                nc.vector.tensor_add(
                                    ps_s[:, :], ps_s[:, :], mask_sb[:, bass.ts(o, 512)])
                            ex = expp.tile([128, 512], MM_DT, tag="ex")
                            nc.scalar.activation(
                                ex[:, :], ps_s[:, :],
                                mybir.ActivationFunctionType.Exp, scale=SCALE)
                            ex_tiles[kj] = ex
                        j = kj - LOOK
                        if j >= 0:
                            nc.tensor.matmul(
                                ps_o[:, :], _r(v_sb[h][:, j, :]), _r(ex_tiles[j][:, :]),
                                start=(j == 0), stop=(j == kmax - 1))
                            nc.tensor.matmul(
                                ps_l[:, :], _r(ones_sb[:, :]), _r(ex_tiles[j][:, :]),
                                start=(j == 0), stop=(j == kmax - 1))
                    # normalize this (h, g) block by 1/l via K=1 broadcast mm
                    lr = lp.tile([1, 512], MM_DT, tag="lr")
                    nc.vector.tensor_copy(lr[:, :], ps_l[:, :])
                    nc.vector.reciprocal(lr[:, :], lr[:, :])
                    ps_lb = pslb.tile([128, 512], F32, tag="plb")
                    nc.tensor.matmul(
                        ps_lb[:, :], onesr_sb[:, :], lr[:, :],
                        start=True, stop=True)
                    at = at_sb[h]
                    nc.vector.tensor_copy(at[:, qcols], ps_o[:, :])
                    nc.vector.tensor_mul(at[:, qcols], at[:, qcols], ps_lb[:, :])

                # out-proj for this q-column chunk, then its ReduceScatter
                for dblk in range(NC):
                    ps = psp3.tile([128, 512], F32, tag="pp")
                    for c4 in range(HPC):
                        nc.tensor.matmul(
                            ps[:, :], _r(wo_sb[:, c4, dblk * 128:(dblk + 1) * 128]),
                            _r(at_sb[c4][:, qcols]),
                            start=(c4 == 0), stop=(c4 == HPC - 1))
                    po_s = post.tile([128, 512], F32, tag="pos")
                    if dblk % 2 == 0:
                        nc.scalar.copy(po_s[:, :], ps[:, :])
                    else:
                        nc.vector.tensor_copy(po_s[:, :], ps[:, :])
                    nc.sync.dma_start(
                        po_sc[g][dblk * 128:(dblk + 1) * 128, :], po_s[:, :])

                nc.gpsimd.collective_compute(
                    "ReduceScatter",
                    mybir.AluOpType.add,
                    replica_groups=GROUPS,
                    ins=[po_sc[g].opt()],
                    outs=[rs_sc[g].opt()],
                )
                nc.gpsimd.dma_start(
                    rsout_d.ap()[:, bass.ts(g, 512)], rs_sc[g][:, :])

    return nc


def _rope_tables():
    freqs = 1.0 / (ROPE_THETA ** (np.arange(0, dh, 2, dtype=np.float32) / dh))
    emb = np.arange(S, dtype=np.float32)[:, None] * freqs[None, :]   # [S, 64]
    cos = np.tile(np.cos(emb)[:, : rope // 2], (1, 2)).T.astype(np.float32)
    sin = np.tile(np.sin(emb)[:, : rope // 2], (1, 2)).T.astype(np.float32)
    sin_signed = sin.copy()
    sin_signed[: rope // 2] *= -1.0
    return np.ascontiguousarray(cos), np.ascontiguousarray(sin_signed)


def _mask_table():
    kk = np.arange(128)[:, None]
    cc = np.arange(512)[None, :]
    cols = [np.where(o * 128 + kk <= cc, 0.0, NEG).astype(np.float32)
            for o in range(NG)]
    return np.ascontiguousarray(np.concatenate(cols, axis=1))


_NC_CACHE = None


def kernel(x, qkv, wk, wo):
    global _NC_CACHE
    if _NC_CACHE is None:
        _NC_CACHE = _build_program()
    nc = _NC_CACHE

    x = np.asarray(x, dtype=np.float32)
    qkv = np.asarray(qkv, dtype=np.float32)
    wk = np.asarray(wk, dtype=np.float32)
    wo = np.asarray(wo, dtype=np.float32)

    cos_t, sin_t = _rope_tables()
    mask_t = _mask_table()

    in_maps = []
    for core in range(NCORES):
        b = core // 4
        hs = (core % 4) * HPC
        in_maps.append({
            "xT": np.ascontiguousarray(x[b].T),
            "wq": np.ascontiguousarray(qkv[hs * dh:(hs + HPC) * dh].T),
            "wkn": np.ascontiguousarray(wk[hs * nope:(hs + HPC) * nope].T),
            "wkr": np.ascontiguousarray(wk[H * nope:].T),
            "wv": np.ascontiguousarray(qkv[D + hs * dh:D + (hs + HPC) * dh].T),
            "wo": np.ascontiguousarray(wo[:, hs * dh:(hs + HPC) * dh].T),
            "cosT": cos_t,
            "sinT": sin_t,
            "maskT": mask_t,
            "ident": np.eye(128, dtype=np.float32),
            "ones": np.ones((128, 1), dtype=np.float32),
            "onesr": np.ones((1, 128), dtype=np.float32),
        })

    res = run_bass_kernel_spmd(nc, in_maps, list(range(NCORES))).results

    out = np.empty((B, S, D), dtype=np.float32)
    k_heads = np.empty((B, H, S, dh), dtype=np.float32)
    v_heads = np.empty((B, H, S, dh), dtype=np.float32)
    for b in range(B):
        poT = np.concatenate(
            [res[b * 4 + r]["out_t_rs"] for r in range(4)], axis=0)   # [D, S]
        out[b] = poT.T
        for r in range(4):
            hs = r * HPC
            k_heads[b, hs:hs + HPC] = res[b * 4 + r]["k_loc"]
            v_heads[b, hs:hs + HPC] = res[b * 4 + r]["v_loc"]
    return out, k_heads, v_heads
